# revision 2
# baseline (speedup 1.0000x reference)
"""AutoRound GPTQ int4 linear on 8 TRN2 NeuronCores — v3.

Same device algorithm as v2 (fp16 dequant + fp16 matmul, fp32 PSUM,
fp16 output), but all four per-core operands are fused into ONE int32
input tensor per core (PJRT-over-axon per-argument dispatch overhead is
a measurable part of steady-state per-execution time):

  blob [4096, 10428] i32 = [ xt(f32->8192 cols) | qweight_rep(1376) |
                             scales_rep(f16->688) | qzeros(172, rows 0..32) ]
"""

import sys

sys.path.insert(0, "/opt/trn_rl_repo")

import numpy as np

import concourse.bacc as bacc
import concourse.mybir as mybir
import concourse.tile as tile
from concourse.bass_utils import run_bass_kernel_spmd

IN_F = 4096
OUT_F = 11008
G = 32  # quant groups (group size 128 == one k-tile)
N_CORES = 8
OUT_SHARD = OUT_F // N_CORES  # 1376
B, S = 4, 2048
M_ROWS = B * S  # 8192
M_BLK = 256

f32 = mybir.dt.float32
f16 = mybir.dt.float16
i32 = mybir.dt.int32
Alu = mybir.AluOpType


def blob_cols(m_rows=M_ROWS, out_shard=OUT_SHARD):
    # [xt | qweight | scales | qzeros]
    return m_rows + out_shard + out_shard // 2 + out_shard // 8


def build_nc(m_rows=M_ROWS, out_shard=OUT_SHARD, in_f=IN_F):
    KT = in_f // 128  # k-tiles; each k-tile is exactly one quant group
    NB = m_rows // M_BLK
    XH = 2  # x DMAs per m-block
    TPD = KT // XH  # k-tiles per x DMA
    assert KT == G and m_rows % M_BLK == 0 and out_shard % 8 == 0

    chunks = []
    o = 0
    while o < out_shard:
        w = min(512, out_shard - o)
        chunks.append((o, w))
        o += w
    n_mt = M_BLK // 128  # m-tiles per block (2)

    XT_C = 0
    QW_C = m_rows
    S_C = QW_C + out_shard
    QZ_C = S_C + out_shard // 2

    nc = bacc.Bacc("TRN2", target_bir_lowering=False)
    blob = nc.dram_tensor(
        "blob", (in_f, blob_cols(m_rows, out_shard)), i32, kind="ExternalInput"
    )
    xt_d = blob[:, XT_C : XT_C + m_rows].bitcast(f32)  # [in_f, m_rows] f32
    qw_d = blob[:, QW_C : QW_C + out_shard]  # [in_f, out_shard] i32
    s_d = blob[:, S_C : S_C + out_shard // 2].bitcast(f16)  # [in_f, out_shard] f16
    qz_d = blob[0:G, QZ_C : QZ_C + out_shard // 8]  # [G, out_shard//8] i32
    out_d = nc.dram_tensor("out", (m_rows, out_shard), f16, kind="ExternalOutput")

    with tile.TileContext(nc) as tc:
        with (
            tc.tile_pool(name="const", bufs=1) as cpool,
            tc.tile_pool(name="wpool", bufs=KT) as wpool,
            tc.tile_pool(name="qrep_p", bufs=2) as qrep_pool,
            tc.tile_pool(name="sb_p", bufs=2) as sb_pool,
            tc.tile_pool(name="row_p", bufs=2) as row_pool,
            tc.tile_pool(name="bcast_p", bufs=2) as bcast_pool,
            tc.tile_pool(name="xb_p", bufs=6) as xb_pool,
            tc.tile_pool(name="out_p", bufs=4) as out_pool,
            tc.tile_pool(name="pout", bufs=8, space="PSUM") as pout_pool,
        ):
            # --- constants ---
            iota_t = cpool.tile([128, 1], i32, tag="iota")
            nc.gpsimd.iota(iota_t[:], pattern=[[0, 1]], base=0, channel_multiplier=4)
            # per-partition nibble shift: 4*(p % 8), int32 tensor operand
            shift_ap = cpool.tile([128, 1], i32, tag="shift")
            nc.vector.tensor_scalar(shift_ap[:], iota_t[:], 28, None, Alu.bitwise_and)

            qz_sb = cpool.tile([G, out_shard // 8], i32, tag="qz_sb")
            nc.sync.dma_start(qz_sb[:], qz_d)
            # unpack zeros along the free dim (int-only: bitvec ops cannot cast)
            z_sbi = cpool.tile([G, out_shard], i32, tag="z_sbi")
            z_r = z_sbi[:].rearrange("g (r i) -> g r i", i=8)
            for i in range(8):
                nc.vector.tensor_scalar(
                    z_r[:, :, i], qz_sb[:], 4 * i, 15,
                    Alu.logical_shift_right, Alu.bitwise_and,
                )
            z_sbh = cpool.tile([G, out_shard], f16, tag="z_sbh")
            nc.vector.tensor_copy(z_sbh[:], z_sbi[:])

            # --- dequantize weight shard into SBUF (fp16, [k, n] layout) ---
            w_tiles = []
            for t in range(KT):
                qrep = qrep_pool.tile([128, out_shard], i32, tag="qrep")
                nc.sync.dma_start(qrep[:], qw_d[128 * t : 128 * (t + 1), :])
                sb = sb_pool.tile([128, out_shard], f16, tag="sb")
                nc.sync.dma_start(sb[:], s_d[128 * t : 128 * (t + 1), :])
                zrow = row_pool.tile([1, out_shard], f16, tag="zrow")
                nc.sync.dma_start(zrow[:], z_sbh[t : t + 1, :])
                zb = bcast_pool.tile([128, out_shard], f16, tag="zb")
                nc.gpsimd.partition_broadcast(zb[:], zrow[:])
                # in-place int chain: q >>= shift; q &= 15
                nc.vector.tensor_tensor(
                    qrep[:], qrep[:],
                    shift_ap[:].broadcast_to((128, out_shard)),
                    Alu.logical_shift_right,
                )
                nc.vector.tensor_scalar(qrep[:], qrep[:], 15, None, Alu.bitwise_and)
                w_t = wpool.tile([128, out_shard], f16, tag="w")
                nc.scalar.copy(w_t[:], qrep[:])  # int32 -> fp16 (values 0..15)
                nc.vector.tensor_tensor(w_t[:], w_t[:], zb[:], Alu.subtract)
                nc.vector.tensor_tensor(w_t[:], w_t[:], sb[:], Alu.mult)
                w_tiles.append(w_t)

            # --- main loop: k-outer over 256-row m-blocks ---
            for mb in range(NB):
                m0 = mb * M_BLK
                xbs = []
                for h in range(XH):
                    xb = xb_pool.tile([128, TPD * M_BLK], f16, tag="xb")
                    src = xt_d[
                        h * TPD * 128 : (h + 1) * TPD * 128, m0 : m0 + M_BLK
                    ].rearrange("(t p) m -> p t m", p=128)
                    dst = xb[:].rearrange("p (t m) -> p t m", m=M_BLK)
                    nc.gpsimd.dma_start(dst, src)  # SWDGE casts f32 -> f16
                    xbs.append(xb)

                pos = [
                    pout_pool.tile([128, w], f32, tag="po", name=f"po_{mb}_{j}_{ci}")
                    for j in range(n_mt)
                    for ci, (o, w) in enumerate(chunks)
                ]
                for t in range(KT):
                    xb = xbs[t // TPD]
                    toff = (t % TPD) * M_BLK
                    for j in range(n_mt):
                        lhs = xb[:, toff + j * 128 : toff + (j + 1) * 128]
                        for ci, (o, w) in enumerate(chunks):
                            nc.tensor.matmul(
                                pos[j * len(chunks) + ci][:],
                                lhs,
                                w_tiles[t][:, o : o + w],
                                start=(t == 0),
                                stop=(t == KT - 1),
                            )
                for j in range(n_mt):
                    outt = out_pool.tile([128, out_shard], f16, tag="outt")
                    for ci, (o, w) in enumerate(chunks):
                        nc.vector.tensor_copy(
                            outt[:, o : o + w], pos[j * len(chunks) + ci][:]
                        )
                    nc.scalar.dma_start(
                        out_d[m0 + j * 128 : m0 + (j + 1) * 128, :], outt[:]
                    )

    nc.compile()
    return nc


_CACHE = {}


def _get_nc():
    if "nc" not in _CACHE:
        _CACHE["nc"] = build_nc()
    return _CACHE["nc"]


def core_blob(xt, qw_shard, qz_shard, sc_shard, m_rows=M_ROWS, out_shard=OUT_SHARD):
    """Pack one core's operands into the fused i32 blob.

    xt: [4096, m_rows] f32 (x transposed), qw_shard: [512, out_shard] i32
    (packed), qz_shard: [G, out_shard//8] i32, sc_shard: [G, out_shard] f16.
    """
    blob = np.zeros((IN_F, blob_cols(m_rows, out_shard)), np.int32)
    c0 = 0
    blob[:, c0 : c0 + m_rows] = xt.view(np.int32)
    c0 += m_rows
    blob[:, c0 : c0 + out_shard] = np.repeat(qw_shard, 8, axis=0)
    c0 += out_shard
    blob[:, c0 : c0 + out_shard // 2] = np.ascontiguousarray(
        np.repeat(sc_shard, 128, axis=0)
    ).view(np.int32)
    c0 += out_shard // 2
    blob[0:G, c0 : c0 + out_shard // 8] = qz_shard
    return blob


def shard_inputs(x, qweight, qzeros, scales):
    x = np.asarray(x, dtype=np.float32).reshape(M_ROWS, IN_F)
    xt = np.ascontiguousarray(x.T)
    qweight = np.asarray(qweight)
    qzeros = np.asarray(qzeros)
    scales = np.asarray(scales)
    pz = OUT_SHARD // 8
    in_maps = []
    for c in range(N_CORES):
        lo, hi = c * OUT_SHARD, (c + 1) * OUT_SHARD
        in_maps.append(
            {
                "blob": core_blob(
                    xt,
                    qweight[:, lo:hi],
                    np.ascontiguousarray(qzeros[:, c * pz : (c + 1) * pz]),
                    scales[:, lo:hi],
                )
            }
        )
    return in_maps


def gather_outputs(results):
    out = np.empty((M_ROWS, OUT_F), np.float32)
    for c in range(N_CORES):
        out[:, c * OUT_SHARD : (c + 1) * OUT_SHARD] = results[c]["out"].astype(
            np.float32
        )
    return out.reshape(B, S, OUT_F)


def kernel(x, qweight, qzeros, scales):
    in_maps = shard_inputs(x, qweight, qzeros, scales)
    res = run_bass_kernel_spmd(_get_nc(), in_maps, core_ids=list(range(N_CORES)))
    return gather_outputs(res.results)
